# revision 2
# baseline (speedup 1.0000x reference)
"""Trainium2 Bass kernel for nn_MultiHeadODELinear.

Math: out = sum_{k=0..4} (t^k/k!) blockdiag(A_h)^k (x @ W.T + b)
Folded:  out = x @ W_eff.T + b_eff,  W_eff = E @ W,  b_eff = E @ b,
  E = blockdiag(M_h),  M_h = sum_k (t^k/k!) A_h^k  (16 heads of 64x64).

Data-parallel over batch: core i handles x[i], supplied TRANSPOSED
(xT [1024, 4096], contiguous) so the contraction dim d is already on
partitions: the main loop is a pure accumulating-matmul stream on the PE
(f32r, 1 cyc/row, 512-wide) with no on-chip transposes at all.

DMA transfers execute one at a time, so the bulk-transfer ORDER is the
schedule (sync ring): t, A, b, then W chunk pairs interleaved with
256-token x blocks (W01, xp0, W23, xp1, W45, xp2, W67, xp3, xp4...).
Out DMAs ride the Activation HWDGE ring.

Phase 0 (all matmuls bf16, 1 cyc/row at any width, ~2e-3 error on W_eff):
  - a PE warmup spin bridges the tensor-engine p-state ramp (~3us to
    full clock) into the first real work;
  - Horner for N = blockdiag(M_h^T) with 2-chunk-wide PSUM->SBUF adds
    (DVE only: GPSIMD cannot access PSUM);
  - WT_eff per 256-col band as W chunk pairs land (W cast f32->bf16 on
    Activation); o-chunk c needs only m-chunk c since N is block-diag;
  - b_eff = N^T b built a 512-half at a time.
Startup runs tiles 0..3 at quarter-band (256-col) granularity following
W-pair / x-block arrival, so the PE computes output while W streams;
steady state uses 512-col bands, bias-add on the DVE copyback, per-band
out DMA.

Measured on HW (8 cores): rel err vs the f32 reference 2.3e-3 (gate
2e-2).  Cost-model timeline: 134.1us single execution (baseline kernel:
163.9us modeled / 176.3us harness-measured).
"""

import sys

for _p in ("/opt/trn_rl_repo",):
    if _p not in sys.path:
        sys.path.insert(0, _p)

import numpy as np

import concourse.bass as bass  # noqa: F401
import concourse.tile as tile
from concourse import bacc, mybir
from concourse import bass_utils
from concourse.masks import make_identity

F32 = mybir.dt.float32
F32R = mybir.dt.float32r
BF16 = mybir.dt.bfloat16

B, S, D = 8, 4096, 1024
H, HD = 16, 64
ORDERS = 4
P = 128
NCHUNK = D // P          # 8 chunks of 128 along any 1024 dim
TTILES = S // P          # 32 token tiles per core
XP = 256                 # tokens per x DMA block
TPB = XP // P            # token tiles per block (2)
NXP = S // XP            # 16 x blocks
N_CORES = 8

_NC_CACHE = {}


def _build_nc(repeats=1, variant=()):
    variant = set(variant)
    nc = bacc.Bacc("TRN2", target_bir_lowering=False, debug=False)

    xt_d = nc.dram_tensor("xT", [D, S], F32R, kind="ExternalInput").ap()
    w_d = nc.dram_tensor("W", [D, D], F32, kind="ExternalInput").ap()
    b_d = nc.dram_tensor("b", [D], F32, kind="ExternalInput").ap()
    a_d = nc.dram_tensor("A", [H, HD, HD], F32, kind="ExternalInput").ap()
    t_d = nc.dram_tensor("t", [P, 1], F32, kind="ExternalInput").ap()
    o_d = nc.dram_tensor("out", [S, D], F32, kind="ExternalOutput").ap()

    xt_v = xt_d.rearrange("(c p) s -> p c s", p=P)

    with tile.TileContext(nc) as tc:
        with tc.tile_pool(name="const", bufs=1) as const_pool, \
             tc.tile_pool(name="wsb", bufs=1) as w_pool, \
             tc.tile_pool(name="xin", bufs=4) as x_pool, \
             tc.tile_pool(name="osb", bufs=6) as o_pool, \
             tc.tile_pool(name="ps_small", bufs=4, space="PSUM") as ps_small, \
             tc.tile_pool(name="ps_o", bufs=2, space="PSUM") as ps_o:

            def stage_a(it):
                xp = x_pool.tile([P, NCHUNK, XP], F32R, tag="xp", name="xp")
                nc.sync.dma_start(xp[:], xt_v[:, :, it * XP:(it + 1) * XP])
                return xp

            if "no_phase0" in variant:
                wte = w_pool.tile([P, NCHUNK, D], F32R, name="wte2")
                nc.gpsimd.memset(wte[:].bitcast(F32), 0.0)
                b_bcast = const_pool.tile([P, D], F32, name="bb2")
                nc.gpsimd.memset(b_bcast[:], 0.0)
            # ---------------- phase 0 + startup schedule ----------------
            early_x = []
            if "no_phase0" not in variant:
                # PE warmup spin: the tensor engine clock ramps to full speed
                # only after ~3us of continuous execution.  A few discarded
                # f32r matmuls bridge t~0.7us to the first real phase-0 work
                # so the (latency-critical) Horner runs at 2.4GHz.
                warm = const_pool.tile([P, 512], F32R, name="warm")
                nc.vector.memset(warm[:].bitcast(F32), 0.0)
                for _ in range(4):
                    ps_wm = ps_small.tile([P, 512], F32, tag="ps0",
                                          name="ps_wm")
                    nc.tensor.matmul(ps_wm[:], warm[:, 0:P], warm[:],
                                     start=True, stop=True)

                # identity first on the Pool ring (cheap, unblocks the c_k*I
                # chain); A lands as f32 early on the sync ring and its
                # diagonal blocks are cast to bf16 on Pool.
                ident = const_pool.tile([P, P], F32)
                make_identity(nc, ident[:])            # Pool

                # sync ring: t, A, b (tiny), then W chunk pairs spread
                # between the x blocks so band work can start early.
                c1 = const_pool.tile([P, 1], F32)
                nc.sync.dma_start(c1[:], t_d[:])
                a_f32 = const_pool.tile([P, NCHUNK, P], F32)
                a_v = a_d.rearrange("(hp two) i j -> two i hp j", two=2)
                nc.sync.dma_start(a_f32[0:HD, :, 0:HD], a_v[0])
                nc.sync.dma_start(a_f32[HD:P, :, HD:P], a_v[1])
                b_f32 = const_pool.tile([P, NCHUNK], F32)
                nc.sync.dma_start(b_f32[:], b_d.rearrange("(c p) -> p c", p=P))
                a_blk = const_pool.tile([P, NCHUNK, P], BF16)
                nc.gpsimd.memset(a_blk[0:HD, :, HD:P], 0.0)
                nc.gpsimd.memset(a_blk[HD:P, :, 0:HD], 0.0)
                nc.gpsimd.tensor_copy(a_blk[0:HD, :, 0:HD],
                                      a_f32[0:HD, :, 0:HD])
                nc.gpsimd.tensor_copy(a_blk[HD:P, :, HD:P],
                                      a_f32[HD:P, :, HD:P])
                w_r = w_pool.tile([P, NCHUNK, D], F32, name="w_r")
                w_b = w_pool.tile([P, NCHUNK, D], BF16, name="w_b")
                w_view = w_d.rearrange("(c p) d -> p c d", p=P)

                def send_w_pair(cp):
                    for c in (2 * cp, 2 * cp + 1):
                        nc.sync.dma_start(w_r[:, c, :], w_view[:, c, :])
                        # cast f32->bf16 on Activation as each chunk lands
                        nc.scalar.mul(w_b[:, c, :], w_r[:, c, :], 1.0)

                send_w_pair(0)
                early_x.append(stage_a(0))
                send_w_pair(1)
                early_x.append(stage_a(1))
                send_w_pair(2)
                early_x.append(stage_a(2))
                send_w_pair(3)
                early_x.append(stage_a(3))

                # c_k = t^k/k! chains on DVE (c1 comes in via the t DMA),
                # then c_k*I constants: c4I narrow bf16 (matmul rhs), c1..c3
                # and ident as 2-chunk-wide f32 tiles for the Horner adds.
                c2 = const_pool.tile([P, 1], F32)
                nc.vector.tensor_scalar(c2[:], c1[:], c1[:], 0.5,
                                        mybir.AluOpType.mult,
                                        mybir.AluOpType.mult)
                c3 = const_pool.tile([P, 1], F32)
                nc.vector.tensor_scalar(c3[:], c2[:], c1[:], 1.0 / 3.0,
                                        mybir.AluOpType.mult,
                                        mybir.AluOpType.mult)
                c4 = const_pool.tile([P, 1], F32)
                nc.vector.tensor_scalar(c4[:], c3[:], c1[:], 0.25,
                                        mybir.AluOpType.mult,
                                        mybir.AluOpType.mult)
                c4I = const_pool.tile([P, P], BF16, tag="cI3")
                nc.vector.tensor_scalar(c4I[:], ident[:], c4[:], None,
                                        mybir.AluOpType.mult)
                addI_wide = []
                for k, ck in enumerate((c3, c2, c1, None)):
                    wI = const_pool.tile([P, 2, P], F32, tag=f"wI{k}")
                    for j in range(2):
                        if ck is None:
                            nc.vector.tensor_copy(wI[:, j, :], ident[:])
                        else:
                            nc.vector.tensor_scalar(wI[:, j, :], ident[:],
                                                    ck[:], None,
                                                    mybir.AluOpType.mult)
                    addI_wide.append(wI)

                # Horner: S <- A_c^T S + c_k I from rhs = c4*I; 4 steps give
                # S = blockdiag(M_h^T) per chunk.  PSUM->SBUF adds run 2
                # chunks wide, ping-ponged DVE / Pool, so each per-group
                # dependency chain is short.
                s_prev = None
                for step in range(ORDERS):
                    s_new = const_pool.tile([P, NCHUNK, P], BF16,
                                            tag=f"S{step}", name=f"S{step}")
                    for grp in range(4):
                        ps_s = ps_small.tile([P, 512], F32, tag="ps0",
                                             name="ps_s")
                        for cc in range(2):
                            c = grp * 2 + cc
                            rhs = c4I[:] if step == 0 else s_prev[:, c, :]
                            nc.tensor.matmul(ps_s[:, cc * P:(cc + 1) * P],
                                             a_blk[:, c, :], rhs,
                                             start=True, stop=True)
                        nc.vector.tensor_tensor(
                            s_new[:, grp * 2:grp * 2 + 2, :],
                            ps_s[:, 0:2 * P], addI_wide[step][:],
                            mybir.AluOpType.add)
                    s_prev = s_new
                n2 = s_prev  # blockdiag(M_h^T), per chunk, bf16

                # WT_eff band build (bf16): o-chunk c needs only m-chunk c.
                wte = w_pool.tile([P, NCHUNK, D], F32R)

                def build_wte_band(cp):
                    # two d-chunks share one PSUM tile; one wide copyback
                    for dp in range(NCHUNK // 2):
                        ps_w = ps_small.tile([P, 512], F32, tag="ps0",
                                             name="ps_w")
                        for sub in range(2):
                            dc = 2 * dp + sub
                            for half in range(2):
                                mc = 2 * cp + half
                                nc.tensor.matmul(
                                    ps_w[:, sub * 2 * P + half * P:
                                         sub * 2 * P + (half + 1) * P],
                                    w_b[:, mc, dc * P:(dc + 1) * P],
                                    n2[:, mc, :], start=True, stop=True)
                        dst = wte[:, 2 * dp:2 * dp + 2,
                                  cp * 2 * P:(cp + 1) * 2 * P]
                        if dp % 2 == 0:
                            nc.scalar.mul(dst, ps_w[:], 1.0)
                        else:
                            nc.vector.tensor_copy(dst, ps_w[:])

                # b_eff = N^T b as a [1, 1024] row, then broadcast; built a
                # 512-col half at a time (half h needs only n2 chunks 4h..).
                ones_b = const_pool.tile([1, P], BF16)
                nc.vector.memset(ones_b[:], 1.0)
                b_sb = const_pool.tile([P, NCHUNK], BF16)
                nc.vector.tensor_copy(b_sb[:], b_f32[:])
                b_row = const_pool.tile([1, D], BF16)
                b_bcast_t = const_pool.tile([P, D], F32, name="b_bcast")

                def build_b_half(half):
                    ps_b = ps_small.tile([P, 512], F32, tag="ps0",
                                         name=f"ps_b{half}")
                    for cc in range(4):
                        c = half * 4 + cc
                        nc.tensor.matmul(ps_b[0:1, cc * P:(cc + 1) * P],
                                         b_sb[:, c:c + 1], n2[:, c, :],
                                         start=True, stop=True)
                    nc.vector.tensor_copy(
                        b_row[:, half * 512:(half + 1) * 512], ps_b[0:1, :])
                    ps_bb = ps_small.tile([P, 512], F32, tag="ps0",
                                          name="ps_bb")
                    nc.tensor.matmul(ps_bb[:], ones_b[:],
                                     b_row[:, half * 512:(half + 1) * 512],
                                     start=True, stop=True)
                    nc.scalar.mul(b_bcast_t[:, half * 512:(half + 1) * 512],
                                  ps_bb[:], 1.0)

            # ---------------- phase 1 ----------------
            n_iters = NXP * repeats
            b_bcast = b_bcast if "no_phase0" in variant else None

            def tile_quarter(xp, tp, tt, cp, o_sb, dma=False):
                # one 256-col quarter-band of one 128-token tile (startup):
                # bias-add per quarter frees PSUM immediately; out DMA per
                # 512 band after its second quarter.
                ps = ps_o.tile([P, 512], F32, tag=f"ps_out{cp % 2}",
                               name=f"ps_out{cp % 2}")
                for dc in range(NCHUNK):
                    nc.tensor.matmul(
                        ps[:, 0:2 * P], xp[:, dc, tp * P:(tp + 1) * P],
                        wte[:, dc, cp * 2 * P:(cp + 1) * 2 * P],
                        start=(dc == 0), stop=(dc == NCHUNK - 1))
                nc.vector.tensor_tensor(
                    o_sb[:, cp * 2 * P:(cp + 1) * 2 * P], ps[:, 0:2 * P],
                    b_bcast[:, cp * 2 * P:(cp + 1) * 2 * P],
                    mybir.AluOpType.add)
                if dma:
                    oh = cp // 2
                    nc.scalar.dma_start(
                        o_d[tt * P:(tt + 1) * P, oh * 512:(oh + 1) * 512],
                        o_sb[:, oh * 512:(oh + 1) * 512])

            def tile_band(xp, tp, tt, oh, o_sb, last=False):
                # one 512-col band of one 128-token tile (steady state)
                ps = ps_o.tile([P, 512], F32, tag=f"ps_out{oh}",
                               name=f"ps_out{oh}")
                for dc in range(NCHUNK):
                    nc.tensor.matmul(
                        ps[:], xp[:, dc, tp * P:(tp + 1) * P],
                        wte[:, dc, oh * 512:(oh + 1) * 512],
                        start=(dc == 0), stop=(dc == NCHUNK - 1))
                nc.vector.tensor_tensor(
                    o_sb[:, oh * 512:(oh + 1) * 512], ps[:],
                    b_bcast[:, oh * 512:(oh + 1) * 512],
                    mybir.AluOpType.add)
                nc.scalar.dma_start(
                    o_d[tt * P:(tt + 1) * P, oh * 512:(oh + 1) * 512],
                    o_sb[:, oh * 512:(oh + 1) * 512])

            def stage_b(it, xp, last=False):
                for tp in range(TPB):
                    tt = (it % NXP) * TPB + tp
                    o_sb = o_pool.tile([P, D], F32, name="o_sb")
                    for oh in range(2):
                        tile_band(xp, tp, tt, oh, o_sb)

            LA = 3  # x block lookahead depth
            if n_iters > 0:
                from collections import deque
                q = deque(early_x)
                next_issue = len(q)
                while next_issue < min(LA, n_iters):
                    q.append(stage_a(next_issue))
                    next_issue += 1
                it0 = 0
                if "no_phase0" not in variant:
                    # startup: tiles 0..3 (x blocks 0,1) at quarter-band
                    # granularity, following W-pair / x-block arrival.
                    xp0 = q.popleft()
                    xp1 = q.popleft()
                    o_sbs = [o_pool.tile([P, D], F32, name="o_sb")
                             for _ in range(4)]
                    build_wte_band(0)
                    b_bcast = b_bcast_t
                    build_b_half(0)
                    for tp in range(TPB):        # tiles 0,1 quarter cp0
                        tile_quarter(xp0, tp, tp, 0, o_sbs[tp])
                    build_wte_band(1)
                    for tp in range(TPB):        # tiles 0,1 cp1 + band-0 DMA
                        tile_quarter(xp0, tp, tp, 1, o_sbs[tp], dma=True)
                    for tp in range(TPB):        # tiles 2,3 band 0
                        tile_quarter(xp1, tp, 2 + tp, 0, o_sbs[2 + tp])
                        tile_quarter(xp1, tp, 2 + tp, 1, o_sbs[2 + tp],
                                     dma=True)
                    build_wte_band(2)
                    build_b_half(1)
                    for tp in range(TPB):        # quarter cp2: tiles 0..3
                        tile_quarter(xp0, tp, tp, 2, o_sbs[tp])
                    for tp in range(TPB):
                        tile_quarter(xp1, tp, 2 + tp, 2, o_sbs[2 + tp])
                    build_wte_band(3)
                    for tp in range(TPB):        # quarter cp3 + band-1 DMA
                        tile_quarter(xp0, tp, tp, 3, o_sbs[tp], dma=True)
                    for tp in range(TPB):
                        tile_quarter(xp1, tp, 2 + tp, 3, o_sbs[2 + tp],
                                     dma=True)
                    it0 = 2
                for it in range(it0, n_iters):
                    while next_issue < min(n_iters, it + LA + 1):
                        q.append(stage_a(next_issue))
                        next_issue += 1
                    stage_b(it, q.popleft(), last=(it == n_iters - 1))

    nc.compile()
    return nc


def get_nc(repeats=1, variant=()):
    key = (repeats, tuple(variant))
    if key not in _NC_CACHE:
        _NC_CACHE[key] = _build_nc(repeats, variant)
    return _NC_CACHE[key]


def make_in_maps(x, t_scalar, W, b, A):
    x = np.asarray(x, dtype=np.float32)
    t = np.asarray(t_scalar, dtype=np.float32)
    t = np.ascontiguousarray(np.broadcast_to(t.reshape(1, 1), (P, 1)))
    W = np.ascontiguousarray(np.asarray(W, dtype=np.float32))
    b = np.ascontiguousarray(np.asarray(b, dtype=np.float32))
    A = np.ascontiguousarray(np.asarray(A, dtype=np.float32))
    return [{"xT": np.ascontiguousarray(x[i].T), "W": W, "b": b, "A": A,
             "t": t} for i in range(N_CORES)]


def kernel(x, t_scalar, W, b, A):
    nc = get_nc()
    in_maps = make_in_maps(x, t_scalar, W, b, A)
    res = bass_utils.run_bass_kernel_spmd(nc, in_maps,
                                          core_ids=list(range(N_CORES)))
    return np.stack([res.results[i]["out"] for i in range(N_CORES)], axis=0)


if __name__ == "__main__":
    rng = np.random.default_rng(0)
    x = rng.standard_normal((B, S, D), dtype=np.float32)
    W = rng.standard_normal((D, D), dtype=np.float32) / 32.0
    b = rng.standard_normal((D,), dtype=np.float32) * 0.01
    A = rng.standard_normal((H, HD, HD), dtype=np.float32) * 0.02
    t = np.float32(0.6)
    out = kernel(x, t, W, b, A)
    print("out", out.shape, out.dtype)


# revision 3
# speedup vs baseline: 1.0081x; 1.0081x over previous
"""Trainium2 Bass kernel for nn_MultiHeadODELinear.

Math: out = sum_{k=0..4} (t^k/k!) blockdiag(A_h)^k (x @ W.T + b)
Folded:  out = x @ W_eff.T + b_eff,  W_eff = E @ W,  b_eff = E @ b,
  E = blockdiag(M_h),  M_h = sum_k (t^k/k!) A_h^k  (16 heads of 64x64).

Data-parallel over batch: core i handles x[i], supplied TRANSPOSED
(xT [1024, 4096], contiguous) so the contraction dim d is already on
partitions: the main loop is a pure accumulating-matmul stream on the PE
(f32r, 1 cyc/row, 512-wide) with no on-chip transposes at all.

DMA transfers execute one at a time, so the bulk-transfer ORDER is the
schedule (sync ring): t, A, b, then W chunk pairs interleaved with
256-token x blocks (W01, xp0, W23, xp1, W45, xp2, W67, xp3, xp4...).
Out DMAs ride the Activation HWDGE ring.

Phase 0 (all matmuls bf16, 1 cyc/row at any width, ~2e-3 error on W_eff):
  - a PE warmup spin bridges the tensor-engine p-state ramp (~3us to
    full clock) into the first real work;
  - Horner for N = blockdiag(M_h^T) with 2-chunk-wide PSUM->SBUF adds
    (DVE only: GPSIMD cannot access PSUM);
  - WT_eff per 256-col band as W chunk pairs land (W cast f32->bf16 on
    Activation); o-chunk c needs only m-chunk c since N is block-diag;
  - b_eff = N^T b built a 512-half at a time.
Startup runs tiles 0..3 at quarter-band (256-col) granularity following
W-pair / x-block arrival, so the PE computes output while W streams;
steady state uses 512-col bands, bias-add on the DVE copyback, per-band
out DMA.

Measured on HW (8 cores): rel err vs the f32 reference 2.3e-3 (gate
2e-2).  Cost-model timeline: 133.0us single execution (baseline kernel:
163.9us modeled / 176.3us harness-measured).
"""

import sys

for _p in ("/opt/trn_rl_repo",):
    if _p not in sys.path:
        sys.path.insert(0, _p)

import numpy as np

import concourse.bass as bass  # noqa: F401
import concourse.tile as tile
from concourse import bacc, mybir
from concourse import bass_utils
from concourse.masks import make_identity

F32 = mybir.dt.float32
F32R = mybir.dt.float32r
BF16 = mybir.dt.bfloat16

B, S, D = 8, 4096, 1024
H, HD = 16, 64
ORDERS = 4
P = 128
NCHUNK = D // P          # 8 chunks of 128 along any 1024 dim
TTILES = S // P          # 32 token tiles per core
XP = 256                 # tokens per x DMA block
TPB = XP // P            # token tiles per block (2)
NXP = S // XP            # 16 x blocks
N_CORES = 8

_NC_CACHE = {}


def _build_nc(repeats=1, variant=()):
    variant = set(variant)
    nc = bacc.Bacc("TRN2", target_bir_lowering=False, debug=False)

    xt_d = nc.dram_tensor("xT", [D, S], F32R, kind="ExternalInput").ap()
    w_d = nc.dram_tensor("W", [D, D], F32, kind="ExternalInput").ap()
    b_d = nc.dram_tensor("b", [D], F32, kind="ExternalInput").ap()
    a_d = nc.dram_tensor("A", [H, HD, HD], F32, kind="ExternalInput").ap()
    t_d = nc.dram_tensor("t", [P, 1], F32, kind="ExternalInput").ap()
    o_d = nc.dram_tensor("out", [S, D], F32, kind="ExternalOutput").ap()

    xt_v = xt_d.rearrange("(c p) s -> p c s", p=P)

    with tile.TileContext(nc) as tc:
        with tc.tile_pool(name="const", bufs=1) as const_pool, \
             tc.tile_pool(name="wsb", bufs=1) as w_pool, \
             tc.tile_pool(name="xin", bufs=4) as x_pool, \
             tc.tile_pool(name="osb", bufs=6) as o_pool, \
             tc.tile_pool(name="ps_small", bufs=4, space="PSUM") as ps_small, \
             tc.tile_pool(name="ps_o", bufs=2, space="PSUM") as ps_o:

            def stage_a(it):
                xp = x_pool.tile([P, NCHUNK, XP], F32R, tag="xp", name="xp")
                nc.sync.dma_start(xp[:], xt_v[:, :, it * XP:(it + 1) * XP])
                return xp

            if "no_phase0" in variant:
                wte = w_pool.tile([P, NCHUNK, D], F32R, name="wte2")
                nc.gpsimd.memset(wte[:].bitcast(F32), 0.0)
                b_bcast = const_pool.tile([P, D], F32, name="bb2")
                nc.gpsimd.memset(b_bcast[:], 0.0)
            # ---------------- phase 0 + startup schedule ----------------
            early_x = []
            if "no_phase0" not in variant:
                # PE warmup spin: the tensor engine clock ramps to full speed
                # only after ~3us of continuous execution.  A few discarded
                # f32r matmuls bridge t~0.7us to the first real phase-0 work
                # so the (latency-critical) Horner runs at 2.4GHz.
                warm = const_pool.tile([P, 512], F32R, name="warm")
                nc.vector.memset(warm[:].bitcast(F32), 0.0)
                for _ in range(4):
                    ps_wm = ps_small.tile([P, 512], F32, tag="ps0",
                                          name="ps_wm")
                    nc.tensor.matmul(ps_wm[:], warm[:, 0:P], warm[:],
                                     start=True, stop=True)

                # identity first on the Pool ring (cheap, unblocks the c_k*I
                # chain); A lands as f32 early on the sync ring and its
                # diagonal blocks are cast to bf16 on Pool.
                ident = const_pool.tile([P, P], F32)
                make_identity(nc, ident[:])            # Pool

                # sync ring: t, A, b (tiny), then W chunk pairs spread
                # between the x blocks so band work can start early.
                a_f32 = const_pool.tile([P, NCHUNK, P], F32)
                a_v = a_d.rearrange("(hp two) i j -> two i hp j", two=2)
                nc.sync.dma_start(a_f32[0:HD, :, 0:HD], a_v[0])
                nc.sync.dma_start(a_f32[HD:P, :, HD:P], a_v[1])
                c1 = const_pool.tile([P, 1], F32)
                nc.sync.dma_start(c1[:], t_d[:])
                b_f32 = const_pool.tile([P, NCHUNK], F32)
                nc.sync.dma_start(b_f32[:], b_d.rearrange("(c p) -> p c", p=P))
                a_blk = const_pool.tile([P, NCHUNK, P], BF16)
                nc.gpsimd.memset(a_blk[0:HD, :, HD:P], 0.0)
                nc.gpsimd.memset(a_blk[HD:P, :, 0:HD], 0.0)
                nc.gpsimd.tensor_copy(a_blk[0:HD, :, 0:HD],
                                      a_f32[0:HD, :, 0:HD])
                nc.gpsimd.tensor_copy(a_blk[HD:P, :, HD:P],
                                      a_f32[HD:P, :, HD:P])
                w_r = w_pool.tile([P, NCHUNK, D], F32, name="w_r")
                w_b = w_pool.tile([P, NCHUNK, D], BF16, name="w_b")
                w_view = w_d.rearrange("(c p) d -> p c d", p=P)

                def send_w_pair(cp):
                    # casts f32->bf16 as each chunk lands; the band-gating
                    # second chunk of pairs 1/3 goes to DVE so both casts of
                    # a pair run in parallel
                    for c in (2 * cp, 2 * cp + 1):
                        nc.sync.dma_start(w_r[:, c, :], w_view[:, c, :])
                        if c in (3, 7):
                            nc.vector.tensor_copy(w_b[:, c, :], w_r[:, c, :])
                        else:
                            nc.scalar.mul(w_b[:, c, :], w_r[:, c, :], 1.0)

                send_w_pair(0)
                early_x.append(stage_a(0))
                send_w_pair(1)
                early_x.append(stage_a(1))
                send_w_pair(2)
                early_x.append(stage_a(2))
                send_w_pair(3)
                early_x.append(stage_a(3))

                # c_k = t^k/k! chains on DVE (c1 comes in via the t DMA),
                # then c_k*I constants: c4I narrow bf16 (matmul rhs), c1..c3
                # and ident as 2-chunk-wide f32 tiles for the Horner adds.
                c2 = const_pool.tile([P, 1], F32)
                nc.vector.tensor_scalar(c2[:], c1[:], c1[:], 0.5,
                                        mybir.AluOpType.mult,
                                        mybir.AluOpType.mult)
                c3 = const_pool.tile([P, 1], F32)
                nc.vector.tensor_scalar(c3[:], c2[:], c1[:], 1.0 / 3.0,
                                        mybir.AluOpType.mult,
                                        mybir.AluOpType.mult)
                c4 = const_pool.tile([P, 1], F32)
                nc.vector.tensor_scalar(c4[:], c3[:], c1[:], 0.25,
                                        mybir.AluOpType.mult,
                                        mybir.AluOpType.mult)
                c4I = const_pool.tile([P, P], BF16, tag="cI3")
                nc.vector.tensor_scalar(c4I[:], ident[:], c4[:], None,
                                        mybir.AluOpType.mult)
                addI_wide = []
                for k, ck in enumerate((c3, c2, c1, None)):
                    wI = const_pool.tile([P, 2, P], F32, tag=f"wI{k}")
                    for j in range(2):
                        if ck is None:
                            nc.vector.tensor_copy(wI[:, j, :], ident[:])
                        else:
                            nc.vector.tensor_scalar(wI[:, j, :], ident[:],
                                                    ck[:], None,
                                                    mybir.AluOpType.mult)
                    addI_wide.append(wI)

                # Horner: S <- A_c^T S + c_k I from rhs = c4*I; 4 steps give
                # S = blockdiag(M_h^T) per chunk.  PSUM->SBUF adds run 2
                # chunks wide, ping-ponged DVE / Pool, so each per-group
                # dependency chain is short.
                s_prev = None
                for step in range(ORDERS):
                    s_new = const_pool.tile([P, NCHUNK, P], BF16,
                                            tag=f"S{step}", name=f"S{step}")
                    for grp in range(4):
                        ps_s = ps_small.tile([P, 512], F32, tag="ps0",
                                             name="ps_s")
                        for cc in range(2):
                            c = grp * 2 + cc
                            rhs = c4I[:] if step == 0 else s_prev[:, c, :]
                            nc.tensor.matmul(ps_s[:, cc * P:(cc + 1) * P],
                                             a_blk[:, c, :], rhs,
                                             start=True, stop=True)
                        nc.vector.tensor_tensor(
                            s_new[:, grp * 2:grp * 2 + 2, :],
                            ps_s[:, 0:2 * P], addI_wide[step][:],
                            mybir.AluOpType.add)
                    s_prev = s_new
                n2 = s_prev  # blockdiag(M_h^T), per chunk, bf16

                # WT_eff band build (bf16): o-chunk c needs only m-chunk c.
                wte = w_pool.tile([P, NCHUNK, D], F32R)

                def build_wte_band(cp):
                    # two d-chunks share one PSUM tile; one wide copyback
                    for dp in range(NCHUNK // 2):
                        ps_w = ps_small.tile([P, 512], F32, tag="ps0",
                                             name="ps_w")
                        for sub in range(2):
                            dc = 2 * dp + sub
                            for half in range(2):
                                mc = 2 * cp + half
                                nc.tensor.matmul(
                                    ps_w[:, sub * 2 * P + half * P:
                                         sub * 2 * P + (half + 1) * P],
                                    w_b[:, mc, dc * P:(dc + 1) * P],
                                    n2[:, mc, :], start=True, stop=True)
                        dst = wte[:, 2 * dp:2 * dp + 2,
                                  cp * 2 * P:(cp + 1) * 2 * P]
                        if dp % 2 == 0:
                            nc.scalar.mul(dst, ps_w[:], 1.0)
                        else:
                            nc.vector.tensor_copy(dst, ps_w[:])

                # b_eff = N^T b as a [1, 1024] row, then broadcast; built a
                # 512-col half at a time (half h needs only n2 chunks 4h..).
                ones_b = const_pool.tile([1, P], BF16)
                nc.vector.memset(ones_b[:], 1.0)
                b_sb = const_pool.tile([P, NCHUNK], BF16)
                nc.vector.tensor_copy(b_sb[:], b_f32[:])
                b_row = const_pool.tile([1, D], BF16)
                b_bcast_t = const_pool.tile([P, D], F32, name="b_bcast")

                def build_b_half(half):
                    ps_b = ps_small.tile([P, 512], F32, tag="ps0",
                                         name=f"ps_b{half}")
                    for cc in range(4):
                        c = half * 4 + cc
                        nc.tensor.matmul(ps_b[0:1, cc * P:(cc + 1) * P],
                                         b_sb[:, c:c + 1], n2[:, c, :],
                                         start=True, stop=True)
                    nc.vector.tensor_copy(
                        b_row[:, half * 512:(half + 1) * 512], ps_b[0:1, :])
                    ps_bb = ps_small.tile([P, 512], F32, tag="ps0",
                                          name="ps_bb")
                    nc.tensor.matmul(ps_bb[:], ones_b[:],
                                     b_row[:, half * 512:(half + 1) * 512],
                                     start=True, stop=True)
                    nc.scalar.mul(b_bcast_t[:, half * 512:(half + 1) * 512],
                                  ps_bb[:], 1.0)

            # ---------------- phase 1 ----------------
            n_iters = NXP * repeats
            b_bcast = b_bcast if "no_phase0" in variant else None

            def tile_quarter(xp, tp, tt, cp, o_sb, dma=False):
                # one 256-col quarter-band of one 128-token tile (startup):
                # bias-add per quarter frees PSUM immediately; out DMA per
                # 512 band after its second quarter.
                ps = ps_o.tile([P, 512], F32, tag=f"ps_out{cp % 2}",
                               name=f"ps_out{cp % 2}")
                for dc in range(NCHUNK):
                    nc.tensor.matmul(
                        ps[:, 0:2 * P], xp[:, dc, tp * P:(tp + 1) * P],
                        wte[:, dc, cp * 2 * P:(cp + 1) * 2 * P],
                        start=(dc == 0), stop=(dc == NCHUNK - 1))
                nc.vector.tensor_tensor(
                    o_sb[:, cp * 2 * P:(cp + 1) * 2 * P], ps[:, 0:2 * P],
                    b_bcast[:, cp * 2 * P:(cp + 1) * 2 * P],
                    mybir.AluOpType.add)
                if dma:
                    oh = cp // 2
                    nc.scalar.dma_start(
                        o_d[tt * P:(tt + 1) * P, oh * 512:(oh + 1) * 512],
                        o_sb[:, oh * 512:(oh + 1) * 512])

            def tile_band(xp, tp, tt, oh, o_sb, last=False):
                # one 512-col band of one 128-token tile (steady state)
                ps = ps_o.tile([P, 512], F32, tag=f"ps_out{oh}",
                               name=f"ps_out{oh}")
                for dc in range(NCHUNK):
                    nc.tensor.matmul(
                        ps[:], xp[:, dc, tp * P:(tp + 1) * P],
                        wte[:, dc, oh * 512:(oh + 1) * 512],
                        start=(dc == 0), stop=(dc == NCHUNK - 1))
                nc.vector.tensor_tensor(
                    o_sb[:, oh * 512:(oh + 1) * 512], ps[:],
                    b_bcast[:, oh * 512:(oh + 1) * 512],
                    mybir.AluOpType.add)
                nc.scalar.dma_start(
                    o_d[tt * P:(tt + 1) * P, oh * 512:(oh + 1) * 512],
                    o_sb[:, oh * 512:(oh + 1) * 512])

            def stage_b(it, xp, last=False):
                for tp in range(TPB):
                    tt = (it % NXP) * TPB + tp
                    o_sb = o_pool.tile([P, D], F32, name="o_sb")
                    for oh in range(2):
                        tile_band(xp, tp, tt, oh, o_sb)

            LA = 3  # x block lookahead depth
            if n_iters > 0:
                from collections import deque
                q = deque(early_x)
                next_issue = len(q)
                while next_issue < min(LA, n_iters):
                    q.append(stage_a(next_issue))
                    next_issue += 1
                it0 = 0
                if "no_phase0" not in variant:
                    # startup: tiles 0..3 (x blocks 0,1) at quarter-band
                    # granularity, following W-pair / x-block arrival.
                    xp0 = q.popleft()
                    xp1 = q.popleft()
                    o_sbs = [o_pool.tile([P, D], F32, name="o_sb")
                             for _ in range(4)]
                    build_wte_band(0)
                    b_bcast = b_bcast_t
                    build_b_half(0)
                    for tp in range(TPB):        # tiles 0,1 quarter cp0
                        tile_quarter(xp0, tp, tp, 0, o_sbs[tp])
                    build_wte_band(1)
                    for tp in range(TPB):        # tiles 0,1 cp1 + band-0 DMA
                        tile_quarter(xp0, tp, tp, 1, o_sbs[tp], dma=True)
                    for tp in range(TPB):        # tiles 2,3 band 0
                        tile_quarter(xp1, tp, 2 + tp, 0, o_sbs[2 + tp])
                        tile_quarter(xp1, tp, 2 + tp, 1, o_sbs[2 + tp],
                                     dma=True)
                    build_wte_band(2)
                    build_b_half(1)
                    for tp in range(TPB):        # quarter cp2: tiles 0..3
                        tile_quarter(xp0, tp, tp, 2, o_sbs[tp])
                    for tp in range(TPB):
                        tile_quarter(xp1, tp, 2 + tp, 2, o_sbs[2 + tp])
                    build_wte_band(3)
                    for tp in range(TPB):        # quarter cp3 + band-1 DMA
                        tile_quarter(xp0, tp, tp, 3, o_sbs[tp], dma=True)
                    for tp in range(TPB):
                        tile_quarter(xp1, tp, 2 + tp, 3, o_sbs[2 + tp],
                                     dma=True)
                    it0 = 2
                for it in range(it0, n_iters):
                    while next_issue < min(n_iters, it + LA + 1):
                        q.append(stage_a(next_issue))
                        next_issue += 1
                    stage_b(it, q.popleft(), last=(it == n_iters - 1))

    nc.compile()
    return nc


def get_nc(repeats=1, variant=()):
    key = (repeats, tuple(variant))
    if key not in _NC_CACHE:
        _NC_CACHE[key] = _build_nc(repeats, variant)
    return _NC_CACHE[key]


def make_in_maps(x, t_scalar, W, b, A):
    x = np.asarray(x, dtype=np.float32)
    t = np.asarray(t_scalar, dtype=np.float32)
    t = np.ascontiguousarray(np.broadcast_to(t.reshape(1, 1), (P, 1)))
    W = np.ascontiguousarray(np.asarray(W, dtype=np.float32))
    b = np.ascontiguousarray(np.asarray(b, dtype=np.float32))
    A = np.ascontiguousarray(np.asarray(A, dtype=np.float32))
    return [{"xT": np.ascontiguousarray(x[i].T), "W": W, "b": b, "A": A,
             "t": t} for i in range(N_CORES)]


def kernel(x, t_scalar, W, b, A):
    nc = get_nc()
    in_maps = make_in_maps(x, t_scalar, W, b, A)
    res = bass_utils.run_bass_kernel_spmd(nc, in_maps,
                                          core_ids=list(range(N_CORES)))
    return np.stack([res.results[i]["out"] for i in range(N_CORES)], axis=0)


if __name__ == "__main__":
    rng = np.random.default_rng(0)
    x = rng.standard_normal((B, S, D), dtype=np.float32)
    W = rng.standard_normal((D, D), dtype=np.float32) / 32.0
    b = rng.standard_normal((D,), dtype=np.float32) * 0.01
    A = rng.standard_normal((H, HD, HD), dtype=np.float32) * 0.02
    t = np.float32(0.6)
    out = kernel(x, t, W, b, A)
    print("out", out.shape, out.dtype)
